# revision 31
# baseline (speedup 1.0000x reference)
"""Multi-head attention kernel for Trainium2, 8 NeuronCores.

Problem: B=4, S=2048, HID=1024, H=16 heads, D=64.
  Q = q@Wq, K = k@Wk, V = v@Wv (reshaped to heads)
  O = softmax(Q K^T / sqrt(D)) V ;  out = O @ Wo

Sharding (hardcoded): core c handles batch b=c//2 and head-half hf=c%2
(8 of 16 heads via column-parallel Wq/Wk/Wv, row-parallel Wo).  Each core
returns a partial output [S, HID]; the host sums the two head-halves per
batch.

Per-core dataflow (all matmuls on PE in float32r, TF32-like):
  phase 1 (three passes K, V, Q): PE-transpose x chunks (fp32 exact,
    identity matmul) -> xT in SBUF; project:
      K^T[e,s] = (Wk blocks)^T @ xT   (e on partitions)
      Q^T[e,s] likewise
      V[s,e]   = (xT blocks)^T @ Wv   (s on partitions, natural layout)
    V is stored bf16 interleaved with a ones column: v_sb[:, st, h, 0:64]=V,
    [...,64]=1.0 so attn@V also produces the softmax row sums.
  phase 2 per (head-pair hp, q-chunk qc): scores S^T[k,q] psum tiles via
    row-packed K=64 matmuls (two heads at partition bases 0/64); ACT exp
    (scale=1/8) drains psum -> P^T bf16 tiles; attn@V accumulates
    O^T[d,q] (+sums row 64) over 16 k-tiles; normalization:
    DVE reciprocal of sums, gpsimd partition-broadcast, DVE multiply ->
    ot_sb float32r.
  phase 3: Y[s,:] accumulated over 4 head-pair e-blocks; DVE drain; DMA out.
"""

import threading

import numpy as np

import concourse.bacc as bacc
import concourse.mybir as mybir
import concourse.tile as tile
from concourse.bass_utils import run_bass_kernel_spmd
from concourse.masks import make_identity

DT = mybir.dt
AF = mybir.ActivationFunctionType

B, S, HID, H = 4, 2048, 1024, 16
D = HID // H               # 64
E = 512                    # local hidden (8 heads)
HLOC = 8                   # heads per core
NHP = 4                    # head pairs per core
SC = 4                     # s-chunks of 512
ST = 16                    # s-tiles of 128
CB = 8                     # contraction blocks of 128 (over HID)
ET = 4                     # e-tiles of 128 in Q^T/K^T
QCW = 512                  # q-chunk width
SCALE = 1.0 / np.sqrt(np.float32(D))   # 0.125

_lock = threading.Lock()
_cache = {}


def _build(debug=False):
    nc = bacc.Bacc(None)
    xq = nc.declare_dram_parameter("xq", [S, HID], DT.float32, isOutput=False)
    xk = nc.declare_dram_parameter("xk", [S, HID], DT.float32, isOutput=False)
    xv = nc.declare_dram_parameter("xv", [S, HID], DT.float32, isOutput=False)
    wq = nc.declare_dram_parameter("wq", [HID, E], DT.float32, isOutput=False)
    wk = nc.declare_dram_parameter("wk", [HID, E], DT.float32, isOutput=False)
    wv = nc.declare_dram_parameter("wv", [HID, E], DT.float32, isOutput=False)
    wo = nc.declare_dram_parameter("wo", [E, HID], DT.float32, isOutput=False)
    y = nc.declare_dram_parameter("y", [S, HID], DT.float32, isOutput=True)
    if debug:
        dbg_qT = nc.declare_dram_parameter("dbg_qT", [128, ET, S], DT.float32, isOutput=True)
        dbg_kT = nc.declare_dram_parameter("dbg_kT", [128, ET, S], DT.float32, isOutput=True)
        dbg_v = nc.declare_dram_parameter("dbg_v", [128, ST, HLOC, D + 1], DT.float32, isOutput=True)
        dbg_ot = nc.declare_dram_parameter("dbg_ot", [128, NHP, S], DT.float32, isOutput=True)

    with tile.TileContext(nc) as tc:
        with (
            tc.tile_pool(name="const", bufs=1) as constp,
            tc.tile_pool(name="wpool", bufs=2) as wpool,
            tc.tile_pool(name="xin", bufs=5) as xinp,
            tc.tile_pool(name="xt", bufs=(1 if debug else 2)) as xtp,
            tc.tile_pool(name="qkv", bufs=1) as qkvp,
            tc.tile_pool(name="pt", bufs=1) as ptp,
            tc.tile_pool(name="norm", bufs=2) as normp,
            tc.tile_pool(name="yout", bufs=2) as youtp,
            tc.tile_pool(name="dbg", bufs=1) as dbgp,
            tc.tile_pool(name="ps1", bufs=2, space="PSUM") as ps1,
            tc.tile_pool(name="ps_s", bufs=2, space="PSUM") as ps_s,
            tc.tile_pool(name="ps_ov", bufs=2, space="PSUM") as ps_ov,
        ):
            ps_p = ps1

            # --- weights: one shared slot tag, loaded per pass ---
            def load_w(wdram):
                wsb = wpool.tile([128, CB, E], DT.float32r, tag="w")
                for cb in range(CB):
                    nc.sync.dma_start(
                        out=wsb[:, cb, :],
                        in_=wdram[cb * 128:(cb + 1) * 128, :].bitcast(DT.float32r),
                    )
                return wsb

            qT = qkvp.tile([128, ET, S], DT.bfloat16, tag="qT")
            kT = qkvp.tile([128, ET, S], DT.bfloat16, tag="kT")
            # V in natural [s, e] layout; per s-tile and head: 64 cols of V
            # plus a ones column (row sums ride the attn@V matmul).
            v_sb = qkvp.tile([128, ST, HLOC, D + 1], DT.bfloat16, tag="v")
            nc.vector.memset(v_sb[:, :, :, D:D + 1], 1.0)

            ident = constp.tile([128, 128], DT.float32)
            make_identity(nc, ident)

            def load_and_transpose(xdram, sc):
                """DMA 4 s-blocks of x, PE-transpose into xt slice [128, CB, 512]."""
                xt_sl = xtp.tile([128, CB, QCW], DT.float32r, tag="xt")
                xins = []
                for sb in range(4):
                    xi = xinp.tile([128, HID], DT.float32, tag="xin")
                    r0 = sc * QCW + sb * 128
                    nc.sync.dma_start(out=xi, in_=xdram[r0:r0 + 128, :])
                    xins.append(xi)
                for cb in range(CB):
                    tp = ps_p.tile([128, QCW], DT.float32, tag="ps1")
                    for sb in range(4):
                        nc.tensor.transpose(
                            tp[:, sb * 128:(sb + 1) * 128],
                            xins[sb][:, cb * 128:(cb + 1) * 128],
                            ident,
                        )
                    nc.vector.tensor_copy(xt_sl[:, cb, :], tp)
                return xt_sl

            # ---------------- pass K then pass V ----------------
            wk_sb = load_w(wk)
            for sc in range(SC):
                xt_sl = load_and_transpose(xk, sc)
                for et in range(ET):
                    pp = ps_p.tile([128, QCW], DT.float32, tag="ps1")
                    for cb in range(CB):
                        nc.tensor.matmul(
                            pp,
                            wk_sb[:, cb, et * 128:(et + 1) * 128],
                            xt_sl[:, cb, :],
                            start=(cb == 0),
                            stop=(cb == CB - 1),
                        )
                    nc.vector.tensor_copy(kT[:, et, sc * QCW:(sc + 1) * QCW], pp)

            wv_sb = load_w(wv)
            for sc in range(SC):
                xt_sl = load_and_transpose(xv, sc)
                for sb in range(4):
                    st = sc * 4 + sb
                    pp = ps_p.tile([128, QCW], DT.float32, tag="ps1")
                    for cb in range(CB):
                        nc.tensor.matmul(
                            pp,
                            xt_sl[:, cb, sb * 128:(sb + 1) * 128],
                            wv_sb[:, cb, :],
                            start=(cb == 0),
                            stop=(cb == CB - 1),
                        )
                    # psum [128s, 512e] -> v_sb[:, st, h, 0:64] for all 8 heads
                    nc.vector.tensor_copy(
                        v_sb[:, st, :, 0:D],
                        pp.rearrange("p (h d) -> p h d", h=HLOC),
                    )

            # ---------------- pass Q + attention rounds ----------------
            wq_sb = load_w(wq)
            # prefetch wo for phase 3 (second wpool slot; overlaps rounds)
            wo_sb = wpool.tile([128, NHP, HID], DT.float32r, tag="w")
            for eb in range(NHP):
                nc.sync.dma_start(
                    out=wo_sb[:, eb, :],
                    in_=wo[eb * 128:(eb + 1) * 128, :].bitcast(DT.float32r),
                )
            for sc in range(SC):
                xt_sl = load_and_transpose(xq, sc)
                for et in range(ET):
                    pp = ps_p.tile([128, QCW], DT.float32, tag="ps1")
                    for cb in range(CB):
                        nc.tensor.matmul(
                            pp,
                            wq_sb[:, cb, et * 128:(et + 1) * 128],
                            xt_sl[:, cb, :],
                            start=(cb == 0),
                            stop=(cb == CB - 1),
                        )
                    nc.vector.tensor_copy(qT[:, et, sc * QCW:(sc + 1) * QCW], pp)

            if debug:
                for et in range(ET):
                    for c4 in range(4):
                        csl = slice(c4 * 512, (c4 + 1) * 512)
                        dq = dbgp.tile([128, 512], DT.float32, tag="dbg")
                        nc.vector.tensor_copy(dq, qT[:, et, csl])
                        nc.sync.dma_start(out=dbg_qT[:, et, csl], in_=dq)
                        dk = dbgp.tile([128, 512], DT.float32, tag="dbg")
                        nc.vector.tensor_copy(dk, kT[:, et, csl])
                        nc.sync.dma_start(out=dbg_kT[:, et, csl], in_=dk)
                for st in range(ST):
                    dv = dbgp.tile([128, HLOC, D + 1], DT.float32, tag="dbg2")
                    nc.vector.tensor_copy(dv, v_sb[:, st, :, :])
                    nc.sync.dma_start(out=dbg_v[:, st, :, :], in_=dv)

            ot_sb = qkvp.tile([128, NHP, S], DT.float32r, tag="ot")
            qsl = slice(0, QCW)

            for qc in range(SC):
                q0 = qc * QCW
                for hp in range(NHP):
                    ovs = [
                        ps_ov.tile([D + 1, QCW], DT.float32, tag="ps_ov",
                                   name=f"ov{qc}_{hp}_{i}")
                        for i in range(2)
                    ]
                    # k-halves keep the live P^T set at 8 k-tiles x 2 heads.
                    # Both heads' score tiles share one 2-bank psum tile so a
                    # single ACT exp drains them (amortizes the ~352-cycle
                    # per-instruction ACT overhead).
                    for kh in range(2):
                        pts = [None] * 8
                        for ki in range(8):
                            kt = kh * 8 + ki
                            sps = ps_s.tile([128, 2, QCW], DT.float32, tag="ps_s")
                            for par in range(2):
                                prow = slice(par * D, par * D + D)
                                nc.tensor.matmul(
                                    sps[:, par, :],
                                    kT[prow, hp, kt * 128:(kt + 1) * 128],
                                    qT[prow, hp, q0:q0 + QCW],
                                    start=True,
                                    stop=True,
                                )
                            ptile = ptp.tile(
                                [128, 2, QCW], DT.bfloat16, tag=f"pt{ki}"
                            )
                            nc.scalar.activation(
                                out=ptile, in_=sps, func=AF.Exp,
                                scale=float(SCALE),
                            )
                            pts[ki] = ptile
                        for ki in range(8):
                            kt = kh * 8 + ki
                            for par in range(2):
                                nc.tensor.matmul(
                                    ovs[par],
                                    v_sb[:, kt, 2 * hp + par, :],
                                    pts[ki][:, par, :],
                                    start=(kt == 0),
                                    stop=(kt == ST - 1),
                                )
                    # normalize: O^T rows /= sums row (row index D).
                    # Drain psum to SBUF immediately (releases the ov bank in
                    # ~1.5us so the next round's attn@V isn't gated on the
                    # whole norm chain), then broadcast/reciprocal/multiply
                    # off the critical path.
                    for par in range(2):
                        ov = ovs[par]
                        sums = normp.tile([1, QCW], DT.float32, tag="sums")
                        nc.vector.tensor_copy(sums, ov[D:D + 1, :])
                        ovst = normp.tile([D, QCW], DT.float32, tag="ovst")
                        nc.vector.tensor_copy(ovst, ov[0:D, :])
                        bc = normp.tile([D, QCW], DT.float32, tag="bc")
                        nc.gpsimd.partition_broadcast(bc, sums)
                        nc.vector.reciprocal(bc, bc)
                        nc.vector.tensor_mul(
                            ot_sb[par * D:par * D + D, hp, q0:q0 + QCW],
                            ovst,
                            bc,
                        )

            if debug:
                for hp in range(NHP):
                    for c4 in range(4):
                        csl = slice(c4 * 512, (c4 + 1) * 512)
                        do = dbgp.tile([128, 512], DT.float32, tag="dbg")
                        nc.vector.tensor_copy(
                            do, ot_sb[:, hp, csl].bitcast(DT.float32))
                        nc.sync.dma_start(out=dbg_ot[:, hp, csl], in_=do)

            # ---------------- output projection ----------------
            for st in range(ST):
                ysb = youtp.tile([128, HID], DT.float32, tag="y")
                for nch in range(2):
                    yp = ps_p.tile([128, QCW], DT.float32, tag="ps1")
                    for hp in range(NHP):
                        nc.tensor.matmul(
                            yp,
                            ot_sb[:, hp, st * 128:(st + 1) * 128],
                            wo_sb[:, hp, nch * QCW:(nch + 1) * QCW],
                            start=(hp == 0),
                            stop=(hp == NHP - 1),
                        )
                    nc.vector.tensor_copy(ysb[:, nch * QCW:(nch + 1) * QCW], yp)
                nc.sync.dma_start(out=y[st * 128:(st + 1) * 128, :], in_=ysb)

    nc.finalize()
    return nc


def _get_nc():
    with _lock:
        if "nc" not in _cache:
            _cache["nc"] = _build()
        return _cache["nc"]


def _in_maps(q, k, v, Wq, Wk, Wv, Wo):
    maps = []
    for c in range(8):
        b, hf = c // 2, c % 2
        cs = slice(hf * E, (hf + 1) * E)
        maps.append({
            "xq": np.ascontiguousarray(q[b]),
            "xk": np.ascontiguousarray(k[b]),
            "xv": np.ascontiguousarray(v[b]),
            "wq": np.ascontiguousarray(Wq[:, cs]),
            "wk": np.ascontiguousarray(Wk[:, cs]),
            "wv": np.ascontiguousarray(Wv[:, cs]),
            "wo": np.ascontiguousarray(Wo[cs, :]),
        })
    return maps


def run(q, k, v, Wq, Wk, Wv, Wo, **spmd_kwargs):
    nc = _get_nc()
    res = run_bass_kernel_spmd(
        nc, _in_maps(q, k, v, Wq, Wk, Wv, Wo), core_ids=list(range(8)),
        **spmd_kwargs,
    )
    out = np.empty((B, S, HID), dtype=np.float32)
    for b in range(B):
        out[b] = res.results[2 * b]["y"] + res.results[2 * b + 1]["y"]
    return out, res


def kernel(q, k, v, Wq, Wk, Wv, Wo):
    out, _ = run(q, k, v, Wq, Wk, Wv, Wo)
    return out
